# revision 1
# baseline (speedup 1.0000x reference)
"""Causal multi-head self-attention with RoPE on 8 Trainium2 NeuronCores.

Problem: b=4, s=2048, d_model=1024, 16 heads, dk=64, causal, RoPE(theta=1e4).

Sharding: 8 cores = (batch, head-half). Core c handles batch c//2 and heads
(c%2)*8 .. +8: QKV projections, causal attention, partial output projection;
the host sums the two partials per batch.

Structure: fine-grained software pipeline over 4 head-pair groups. The Q/K
projections and RoPE for group g+1 are emitted in four slices interleaved
between the attention chunks of group g, so the prep finishes inside the
scalar-bound attention window and the next group starts without a stall.
RoPE is applied per 512-token slice as soon as its projection chunk lands.
Scores are row-packed matmul pairs (two dk=64 heads in distinct PE row
groups, concurrent); PV is a col-packed pair writing head A to partitions
0-63 and head B to 64-127; softmax denominators ride M=1 col-tiled matmuls
against a ones column. Projections hold each stationary weight for two
matmuls (two live PSUM accumulators) to halve the LDWEIGHTS tax; Wo shares
each lhs stationary across both output halves. Softmax 1/sum runs on an
8-wide repacked tile (DVE reciprocal is 8 cycles/element) and broadcasts via
a tiny DRAM bounce. Everything is bf16 except PSUM and the denominators.
"""
import sys
import numpy as np

for _p in ('/root/.axon_site/_ro/trn_rl_repo', '/opt/trn_rl_repo'):
    if _p not in sys.path:
        sys.path.append(_p)

import concourse.bass as bass
import concourse.tile as tile
from concourse import bacc, mybir
from concourse.bass_utils import run_bass_kernel_spmd

F32 = mybir.dt.float32
BF16 = mybir.dt.bfloat16
EXP = mybir.ActivationFunctionType.Exp
MUL = mybir.AluOpType.mult

B, S, D = 4, 2048, 1024
NH, DK = 16, 64
NHC = 8            # heads per core
HD = NHC * DK      # 512
NG = 4             # head-pairs per core
NC = 512           # q-chunk
N_CHUNKS = S // NC
N_KT = S // 128
KSUB = D // 128
THETA = 10000.0

_CACHED = {}


def _build():
    nc = bacc.Bacc('TRN2', target_bir_lowering=False, debug=False, num_devices=8)
    xT = nc.dram_tensor('xT', [D, S], BF16, kind='ExternalInput').ap()
    wqT = nc.dram_tensor('wqT', [D, HD], BF16, kind='ExternalInput').ap()
    wkT = nc.dram_tensor('wkT', [D, HD], BF16, kind='ExternalInput').ap()
    wvT = nc.dram_tensor('wvT', [D, HD], BF16, kind='ExternalInput').ap()
    woT = nc.dram_tensor('woT', [HD, D], BF16, kind='ExternalInput').ap()
    cosd = nc.dram_tensor('cosd', [128, S], BF16, kind='ExternalInput').ap()
    sind = nc.dram_tensor('sind', [128, S], BF16, kind='ExternalInput').ap()
    maskd = nc.dram_tensor('maskd', [128, 128], BF16, kind='ExternalInput').ap()
    y = nc.dram_tensor('y', [S, D], BF16, kind='ExternalOutput').ap()
    recip_d = nc.dram_tensor('recip_d', [NG, N_CHUNKS, 2, NC], BF16).ap()

    with tile.TileContext(nc) as tc:
        with tc.tile_pool(name='persist', bufs=1) as persist, \
             tc.tile_pool(name='qk', bufs=2) as qkpool, \
             tc.tile_pool(name='wqk', bufs=24) as wqkpool, \
             tc.tile_pool(name='rope', bufs=3) as ropep, \
             tc.tile_pool(name='pp', bufs=6) as pp, \
             tc.tile_pool(name='dent', bufs=2) as dent, \
             tc.tile_pool(name='rct', bufs=2) as rct, \
             tc.tile_pool(name='ytp', bufs=4) as ytp, \
             tc.tile_pool(name='scps', bufs=2, space='PSUM') as scps, \
             tc.tile_pool(name='pvps', bufs=1, space='PSUM') as pvps, \
             tc.tile_pool(name='denps', bufs=1, space='PSUM') as denps, \
             tc.tile_pool(name='projps', bufs=2, space='PSUM') as projps:

            qk_wts = {}     # (g, 0/1) -> list of 8 weight tiles
            qk_tiles = {}   # g -> [q_tile, k_tile]

            def load_w(g, which):
                key = (g, which)
                if key not in qk_wts:
                    w_ap = wqT if which == 0 else wkT
                    nm = 'wq' if which == 0 else 'wk'
                    wts = []
                    for s in range(KSUB):
                        wt = wqkpool.tile([128, 128], BF16, tag='w',
                                          name=f'{nm}{g}_{s}')
                        nc.sync.dma_start(
                            wt[:], w_ap[128 * s:128 * (s + 1),
                                        128 * g:128 * (g + 1)])
                        wts.append(wt)
                    qk_wts[key] = wts
                return qk_wts[key]

            # group-0 Q/K weights go first so the first projection matmuls
            # are not queued behind the bulk x/wv/wo loads
            load_w(0, 0)
            load_w(0, 1)

            x_sb = []
            for s in range(KSUB):
                x_sb.append(persist.tile([128, S], BF16, tag=f'x{s}',
                                         name=f'x{s}'))
            cos_sb = persist.tile([128, S], BF16, tag='cos_sb')
            sin_sb = persist.tile([128, S], BF16, tag='sin_sb')
            for c in range(N_CHUNKS):
                for s in range(KSUB):
                    nc.sync.dma_start(x_sb[s][:, NC * c:NC * (c + 1)],
                                      xT[128 * s:128 * (s + 1),
                                         NC * c:NC * (c + 1)])
                if c == 0:   # RoPE tables ahead of the bulk x columns
                    nc.sync.dma_start(cos_sb[:], cosd)
                    nc.sync.dma_start(sin_sb[:], sind)
            v_sb = persist.tile([128, N_KT, NHC, DK], BF16, tag='v_sb')
            lhs_sb = persist.tile([128, NG, S], BF16, tag='lhs_sb')
            tri_sb = persist.tile([128, 128], BF16, tag='tri_sb')
            nc.sync.dma_start(tri_sb[:], maskd)
            ones_sb = persist.tile([128, 1], BF16, tag='ones_sb')
            nc.vector.memset(ones_sb[:], 1.0)
            wo_sb = persist.tile([128, NG, D], BF16, tag='wo_sb')
            for g in range(NG):
                nc.sync.dma_start(wo_sb[:, g], woT[128 * g:128 * (g + 1), :])
            wv_tiles = []
            for s in range(KSUB):
                wt = persist.tile([128, HD], BF16, tag=f'wv{s}')
                nc.sync.dma_start(wt[:], wvT[128 * s:128 * (s + 1), :])
                wv_tiles.append(wt)

            def emit_proj_pair(g, which, c0, evac_engine):
                """Projection of chunks (c0, c0+1) for Q (which=0) / K (1).

                Both chunks accumulate concurrently so each stationary w tile
                is loaded once per two matmuls."""
                wts = load_w(g, which)
                dst = qk_tiles[g][which]
                ps = [projps.tile([128, NC], F32, tag='proj',
                                  name=f'pqk{g}_{which}_{c0}_{i}')
                      for i in range(2)]
                for s in range(KSUB):
                    for i in range(2):
                        nc.tensor.matmul(
                            ps[i][:], wts[s][:],
                            x_sb[s][:, NC * (c0 + i):NC * (c0 + i + 1)],
                            start=(s == 0), stop=(s == KSUB - 1))
                for i in range(2):
                    dcol = slice(NC * (c0 + i), NC * (c0 + i + 1))
                    if evac_engine == 'scalar':
                        nc.scalar.copy(dst[:, dcol], ps[i][:])
                    else:
                        nc.vector.tensor_copy(dst[:, dcol], ps[i][:])

            def emit_rope_slice(g, which, c):
                """In-place RoPE on one 512-token slice of q/k (bf16)."""
                t_sb = qk_tiles[g][which]
                cs = slice(NC * c, NC * (c + 1))
                sw = ropep.tile([128, NC], BF16, tag='sw')
                for blk in range(4):
                    src = (blk // 2) * 64 + (1 - blk % 2) * 32
                    nc.sync.dma_start(sw[32 * blk:32 * (blk + 1), :],
                                      t_sb[src:src + 32, cs])
                t1 = ropep.tile([128, NC], BF16, tag='t1')
                nc.vector.tensor_mul(t1[:], t_sb[:, cs], cos_sb[:, cs])
                t2 = ropep.tile([128, NC], BF16, tag='t2')
                nc.gpsimd.tensor_tensor(t2[:], sw[:], sin_sb[:, cs], MUL)
                nc.vector.tensor_add(t_sb[:, cs], t1[:], t2[:])

            def emit_prep(g, step, evac_engine='vector'):
                """One quarter of group g's Q/K prep: a projection chunk-pair
                plus RoPE on the two finished slices."""
                if step == 0:
                    qk_tiles[g] = [
                        qkpool.tile([128, S], BF16, tag='q', name=f'q{g}'),
                        qkpool.tile([128, S], BF16, tag='k', name=f'k{g}')]
                which, c0 = ((0, 0), (1, 0), (0, 2), (1, 2))[step]
                emit_proj_pair(g, which, c0, evac_engine)
                emit_rope_slice(g, which, c0)
                emit_rope_slice(g, which, c0 + 1)

            def emit_v_proj(t0, t1):
                """V natural layout [tok, t, h, dk] bf16, all 8 heads."""
                for t in range(t0, t1):
                    ps = projps.tile([128, NC], F32, tag='proj', name=f'pv{t}')
                    for s in range(KSUB):
                        nc.tensor.matmul(
                            ps[:], x_sb[s][:, 128 * t:128 * (t + 1)],
                            wv_tiles[s][:],
                            start=(s == 0), stop=(s == KSUB - 1))
                    nc.scalar.copy(v_sb[:, t],
                                   ps.rearrange('p (h m) -> p h m', h=NHC))

            def emit_wo_chunk(c):
                """Output projection for the 4 token tiles of chunk c.

                Both halves accumulate concurrently so each lhs stationary is
                loaded once per two matmuls."""
                for t in range(4 * c, 4 * (c + 1)):
                    ts_ = slice(128 * t, 128 * (t + 1))
                    ps = [projps.tile([128, NC], F32, tag='proj',
                                      name=f'pwo{t}_{i}') for i in range(2)]
                    for g in range(NG):
                        for half in range(2):
                            nc.tensor.matmul(
                                ps[half][:], lhs_sb[:, g, ts_],
                                wo_sb[:, g, NC * half:NC * (half + 1)],
                                start=(g == 0), stop=(g == NG - 1))
                    for half in range(2):
                        yt = ytp.tile([128, NC], BF16, tag='yt')
                        nc.vector.tensor_copy(yt[:], ps[half][:])
                        nc.sync.dma_start(y[ts_, NC * half:NC * (half + 1)],
                                          yt[:])

            def emit_att_chunk(g, c):
                    qt, kt_ = qk_tiles[g]
                    hA, hB = 2 * g, 2 * g + 1
                    cs = slice(NC * c, NC * (c + 1))
                    pv = pvps.tile([128, NC], F32, tag='pv')
                    den = denps.tile([128, NC], F32, tag='den')
                    n_kt = 4 * (c + 1)
                    for kt in range(n_kt):
                        ks = slice(128 * kt, 128 * (kt + 1))
                        j = kt - 4 * c          # >=0 on the diagonal
                        v0 = max(j, 0) * 128    # first valid q column
                        sA = slice(v0, NC)
                        sB = slice(NC + v0, 2 * NC)
                        st_, sp_ = (kt == 0), (kt == n_kt - 1)
                        qs = slice(NC * c + v0, NC * (c + 1))
                        w = NC - v0
                        sc = scps.tile([128, 2 * NC], F32, tag='sc')
                        nc.tensor.matmul(sc[:, sA], kt_[0:64, ks],
                                         qt[0:64, qs], start=True, stop=True)
                        nc.tensor.matmul(sc[:, sB], kt_[64:128, ks],
                                         qt[64:128, qs], start=True, stop=True)
                        p = pp.tile([128, 2 * NC], BF16, tag='p')
                        # one exp over both heads' valid columns (2-segment AP)
                        sc_seg = bass.AP(tensor=sc.tensor, offset=sc.offset + v0,
                                         ap=[list(sc.ap[0]), [NC, 2], [1, w]])
                        p_seg = bass.AP(tensor=p.tensor, offset=p.offset + v0,
                                        ap=[list(p.ap[0]), [NC, 2], [1, w]])
                        nc.scalar.activation(out=p_seg, in_=sc_seg, func=EXP,
                                             scale=1.0 / np.sqrt(DK))
                        if j >= 0:               # triangle on the diag sub-block
                            dseg = bass.AP(
                                tensor=p.tensor, offset=p.offset + v0,
                                ap=[list(p.ap[0]), [NC, 2], [1, 128]])
                            nc.vector.tensor_tensor(
                                dseg, dseg,
                                tri_sb[:, None, :].to_broadcast([128, 2, 128]),
                                MUL)
                        nc.tensor.matmul(pv[0:64, sA], v_sb[:, kt, hA, :],
                                         p[:, sA], start=st_, stop=sp_)
                        nc.tensor.matmul(pv[64:128, sA], v_sb[:, kt, hB, :],
                                         p[:, sB], start=st_, stop=sp_)
                        nc.tensor.matmul(den[64:65, sA], ones_sb[:],
                                         p[:, sA], start=st_, stop=sp_,
                                         tile_position=(0, 64))
                        nc.tensor.matmul(den[96:97, sA], ones_sb[:],
                                         p[:, sB], start=st_, stop=sp_,
                                         tile_position=(0, 96))
                    # chunk end: 1/den on an 8-wide repack, broadcast, evac
                    stage = dent.tile([128, NC], BF16, tag='dstage')
                    nc.vector.tensor_copy(stage[64:97, :], den[64:97, :])
                    packed = dent.tile([128, 8], BF16, tag='dpack')
                    nc.sync.dma_start(packed[0:64, :], stage[64:65, :])
                    nc.sync.dma_start(packed[64:128, :], stage[96:97, :])
                    packr = dent.tile([128, 8], BF16, tag='dpackr')
                    with nc.allow_low_precision(reason='softmax 1/sum in bf16'):
                        nc.vector.reciprocal(packr[:], packed[:])
                    nc.sync.dma_start(recip_d[g, c], packr[:])
                    rc = rct.tile([128, NC], BF16, tag='rc')
                    for half in range(2):
                        nc.sync.dma_start(
                            rc[64 * half:64 * (half + 1), :],
                            bass.AP(tensor=recip_d.tensor,
                                    offset=recip_d.offset
                                    + ((g * N_CHUNKS + c) * 2 + half) * NC,
                                    ap=[[0, 64], [1, NC]]))
                    # fast copy-evac frees the single pv bank; normalize
                    # in place once rc lands (bf16 SBUF-only, 2x DVE mode)
                    nc.vector.tensor_copy(lhs_sb[:, g, cs], pv[:])
                    nc.vector.tensor_tensor(lhs_sb[:, g, cs],
                                            lhs_sb[:, g, cs], rc[:], MUL)

            # ------------- pipeline -------------
            # startup: attention chunks of group 0 interleave with the V
            # projection quads and the remaining group-0 prep slices, so exp
            # work starts as early as possible
            emit_prep(0, 0, 'scalar')
            emit_prep(0, 1, 'scalar')
            emit_v_proj(0, 4)
            emit_att_chunk(0, 0)
            emit_v_proj(4, 8)
            emit_prep(0, 2, 'scalar')
            emit_att_chunk(0, 1)
            emit_prep(0, 3, 'scalar')
            emit_v_proj(8, 12)
            emit_att_chunk(0, 2)
            emit_v_proj(12, 16)
            emit_prep(1, 0)
            emit_att_chunk(0, 3)
            emit_prep(1, 1)
            for g in range(1, NG):
                for c in range(N_CHUNKS):
                    emit_att_chunk(g, c)
                    if c < 2:
                        emit_prep(g, c + 2)       # finish own prep (q/k 2,3)
                    elif g < NG - 1:
                        emit_prep(g + 1, c - 2)   # next group's prep (q/k 0,1)
                    if g == NG - 1:
                        emit_wo_chunk(c)
    nc.compile()
    return nc


def _host_inputs(x, Wq, Wk, Wv, Wo, token_positions):
    """Per-core input maps (host-side sharding / layout / dtype prep only)."""
    import ml_dtypes
    bf16 = ml_dtypes.bfloat16
    perm = np.empty(DK, np.int64)
    perm[0:32] = np.arange(0, DK, 2)
    perm[32:64] = np.arange(1, DK, 2)

    inv_freq = 1.0 / (THETA ** (np.arange(0, DK, 2, dtype=np.float64) / DK))  # [32]
    ang = token_positions.astype(np.float64)[None, :] * inv_freq[:, None]     # [32, S]
    cos32 = np.cos(ang).astype(np.float32)
    sin32 = np.sin(ang).astype(np.float32)
    cos128 = np.tile(cos32, (4, 1)).astype(bf16)
    sin128 = np.concatenate([-sin32, sin32, -sin32, sin32], axis=0).astype(bf16)

    tri = (np.arange(128)[None, :] >= np.arange(128)[:, None]).astype(bf16)

    in_maps = []
    for core in range(8):
        b = core // 2
        h0 = (core % 2) * NHC
        cols = slice(h0 * DK, (h0 + NHC) * DK)
        wq_s = Wq[cols, :].reshape(NHC, DK, D)[:, perm, :].reshape(HD, D)
        wk_s = Wk[cols, :].reshape(NHC, DK, D)[:, perm, :].reshape(HD, D)
        in_maps.append({
            'xT': np.ascontiguousarray(x[b].T.astype(bf16)),
            'wqT': np.ascontiguousarray(wq_s.T.astype(bf16)),
            'wkT': np.ascontiguousarray(wk_s.T.astype(bf16)),
            'wvT': np.ascontiguousarray(Wv[cols, :].T.astype(bf16)),
            'woT': np.ascontiguousarray(Wo[:, cols].T.astype(bf16)),
            'cosd': cos128, 'sind': sin128, 'maskd': tri,
        })
    return in_maps


def kernel(x, Wq, Wk, Wv, Wo, token_positions, _results_hook=None):
    if 'nc' not in _CACHED:
        _CACHED['nc'] = _build()
    nc = _CACHED['nc']
    in_maps = _host_inputs(np.asarray(x), np.asarray(Wq), np.asarray(Wk),
                           np.asarray(Wv), np.asarray(Wo),
                           np.asarray(token_positions))
    res = run_bass_kernel_spmd(nc, in_maps, list(range(8)),
                               **(_results_hook or {}))
    if _results_hook is not None:
        _CACHED['last'] = res
    out = np.empty((B, S, D), np.float32)
    for b in range(B):
        out[b] = (res.results[2 * b]['y'].astype(np.float32)
                  + res.results[2 * b + 1]['y'].astype(np.float32))
    return out



# revision 2
# speedup vs baseline: 1.0051x; 1.0051x over previous
"""Causal multi-head self-attention with RoPE on 8 Trainium2 NeuronCores.

Problem: b=4, s=2048, d_model=1024, 16 heads, dk=64, causal, RoPE(theta=1e4).

Sharding: 8 cores = (batch, head-half). Core c handles batch c//2 and heads
(c%2)*8 .. +8: QKV projections, causal attention, partial output projection;
the host sums the two partials per batch.

V2 structure: chunk-major attention (all 4 head-pair groups per q-chunk, then
the output projection for that chunk) with a fine-grained projection "feeder".
The Q/K/V/Wo projection matmuls are sliced into ~1us closures and pulled into
the PE instruction stream between attention blocks, so the exp latency on the
scalar engine hides behind independent projection work. Scores for block kt
are emitted before PV of block kt-1 (one-block softmax lookahead). Softmax
denominators ride ones-column matmuls packed 4-wide across kt pairs (PE
column groups 0/32/64/96), halving their wall cost. RoPE runs per projection
chunk-pair with one 4-block partition-swap DMA set per 1024 tokens. The exp
activation table is pre-warmed during the input DMA window.
"""
import sys
import numpy as np

for _p in ('/root/.axon_site/_ro/trn_rl_repo', '/opt/trn_rl_repo'):
    if _p not in sys.path:
        sys.path.append(_p)

import concourse.bass as bass
import concourse.tile as tile
from concourse import bacc, mybir
from concourse.bass_utils import run_bass_kernel_spmd

F32 = mybir.dt.float32
BF16 = mybir.dt.bfloat16
EXP = mybir.ActivationFunctionType.Exp
MUL = mybir.AluOpType.mult

B, S, D = 4, 2048, 1024
NH, DK = 16, 64
NHC = 8            # heads per core
HD = NHC * DK      # 512
NG = 4             # head-pairs per core
NC = 512           # q-chunk
N_CHUNKS = S // NC
N_KT = S // 128
KSUB = D // 128
THETA = 10000.0

_CACHED = {}


def _build():
    nc = bacc.Bacc('TRN2', target_bir_lowering=False, debug=False, num_devices=8)
    xT = nc.dram_tensor('xT', [D, S], BF16, kind='ExternalInput').ap()
    wqT = nc.dram_tensor('wqT', [D, HD], BF16, kind='ExternalInput').ap()
    wkT = nc.dram_tensor('wkT', [D, HD], BF16, kind='ExternalInput').ap()
    wvT = nc.dram_tensor('wvT', [D, HD], BF16, kind='ExternalInput').ap()
    woT = nc.dram_tensor('woT', [HD, D], BF16, kind='ExternalInput').ap()
    cosd = nc.dram_tensor('cosd', [128, S], BF16, kind='ExternalInput').ap()
    sind = nc.dram_tensor('sind', [128, S], BF16, kind='ExternalInput').ap()
    maskd = nc.dram_tensor('maskd', [128, 128], BF16, kind='ExternalInput').ap()
    y = nc.dram_tensor('y', [S, D], BF16, kind='ExternalOutput').ap()
    recip_d = nc.dram_tensor('recip_d', [NG, N_CHUNKS, 2, NC], BF16).ap()

    with tile.TileContext(nc) as tc:
        with tc.tile_pool(name='persist', bufs=1) as persist, \
             tc.tile_pool(name='rope', bufs=2) as ropep, \
             tc.tile_pool(name='pp', bufs=6) as pp, \
             tc.tile_pool(name='dent', bufs=2) as dent, \
             tc.tile_pool(name='rct', bufs=2) as rct, \
             tc.tile_pool(name='ytp', bufs=4) as ytp, \
             tc.tile_pool(name='scps', bufs=2, space='PSUM') as scps, \
             tc.tile_pool(name='pvps', bufs=1, space='PSUM') as pvps, \
             tc.tile_pool(name='denps', bufs=1, space='PSUM') as denps, \
             tc.tile_pool(name='projps', bufs=2, space='PSUM') as projps:

            # ---------- persistent tiles; DMA issue order = arrival order ----
            wqk = {}

            def load_w(g, which):
                w_ap = wqT if which == 0 else wkT
                nm = 'wq' if which == 0 else 'wk'
                wts = []
                for s in range(KSUB):
                    wt = persist.tile([128, 128], BF16, tag=f'{nm}{g}_{s}',
                                      name=f'{nm}{g}_{s}')
                    nc.sync.dma_start(
                        wt[:], w_ap[128 * s:128 * (s + 1),
                                    128 * g:128 * (g + 1)])
                    wts.append(wt)
                wqk[(g, which)] = wts

            x_sb = []
            for s in range(KSUB):
                x_sb.append(persist.tile([128, S], BF16, tag=f'x{s}',
                                         name=f'x{s}'))
            cos_sb = persist.tile([128, S], BF16, tag='cos_sb')
            sin_sb = persist.tile([128, S], BF16, tag='sin_sb')
            tri_sb = persist.tile([128, 128], BF16, tag='tri_sb')
            ones_sb = persist.tile([128, 1], BF16, tag='ones_sb')
            v_sb = persist.tile([128, N_KT, NHC, DK], BF16, tag='v_sb')
            lhs_sb = persist.tile([128, NG, S], BF16, tag='lhs_sb')
            wo_sb = persist.tile([128, NG, D], BF16, tag='wo_sb')
            wv_tiles = []
            for s in range(KSUB):
                wv_tiles.append(persist.tile([128, HD], BF16, tag=f'wv{s}',
                                             name=f'wv{s}'))

            def dma_x(c):
                for s in range(KSUB):
                    nc.sync.dma_start(x_sb[s][:, NC * c:NC * (c + 1)],
                                      xT[128 * s:128 * (s + 1),
                                         NC * c:NC * (c + 1)])

            # startup-critical first: g0 Q/K weights, x chunk 0, V weights,
            # RoPE tables for the first half, the causal mask
            load_w(0, 0)
            load_w(0, 1)
            dma_x(0)
            for s in range(KSUB):
                nc.sync.dma_start(wv_tiles[s][:],
                                  wvT[128 * s:128 * (s + 1), :])
            nc.sync.dma_start(cos_sb[:, 0:2 * NC], cosd[:, 0:2 * NC])
            nc.sync.dma_start(sin_sb[:, 0:2 * NC], sind[:, 0:2 * NC])
            nc.sync.dma_start(tri_sb[:], maskd)
            nc.vector.memset(ones_sb[:], 1.0)
            # pre-warm the exp activation table during the DMA window
            wrm = dent.tile([128, 8], BF16, tag='warm')
            wrm2 = dent.tile([128, 8], BF16, tag='warm2')
            nc.vector.memset(wrm[:], 0.0)
            nc.scalar.activation(out=wrm2[:], in_=wrm[:], func=EXP, scale=1.0)
            dma_x(1)
            load_w(1, 0)
            load_w(1, 1)
            dma_x(2)
            dma_x(3)
            nc.sync.dma_start(cos_sb[:, 2 * NC:S], cosd[:, 2 * NC:S])
            nc.sync.dma_start(sin_sb[:, 2 * NC:S], sind[:, 2 * NC:S])
            load_w(2, 0)
            load_w(2, 1)
            load_w(3, 0)
            load_w(3, 1)
            for g in range(NG):
                nc.sync.dma_start(wo_sb[:, g], woT[128 * g:128 * (g + 1), :])

            qk_tiles = {}
            for g in range(NG):
                qk_tiles[g] = [
                    persist.tile([128, S], BF16, tag=f'q{g}', name=f'q{g}'),
                    persist.tile([128, S], BF16, tag=f'k{g}', name=f'k{g}')]

            # ---------- feeder: projection work sliced into ~1us closures ----
            feeder = []          # list of [tag_or_None, cost_ns, closure]
            done_tags = set()
            state = {'ns': 0.0, 'kts': 4 * (1 + 2 + 3 + 4) * NG // NG}
            state['kts'] = sum(4 * (c + 1) for c in range(N_CHUNKS)) * NG

            def push(tag, closures):
                for i, (cost, fn) in enumerate(closures):
                    t = tag if i == len(closures) - 1 else None
                    feeder.append((t, cost, fn))
                    state['ns'] += cost

            def run_next():
                t, cost, fn = feeder.pop(0)
                fn()
                state['ns'] -= cost
                if t is not None:
                    done_tags.add(t)

            def drain(tag):
                while tag not in done_tags:
                    assert feeder, f'feeder empty before {tag}'
                    run_next()

            def pull(budget):
                while feeder and budget > 0:
                    budget -= feeder[0][1]
                    run_next()

            def prep_item(g, which, cp, evac):
                """Q/K projection + RoPE for chunks (cp, cp+1) of group g."""
                wts = wqk[(g, which)]
                dst = qk_tiles[g][which]
                cs2 = slice(NC * cp, NC * (cp + 2))
                box = {}

                def mk_mm(s0):
                    def fn():
                        if s0 == 0:
                            box['ps'] = [
                                projps.tile([128, NC], F32, tag='proj',
                                            name=f'pqk{g}_{which}_{cp}_{i}')
                                for i in range(2)]
                        for s in (s0, s0 + 1):
                            for i in range(2):
                                nc.tensor.matmul(
                                    box['ps'][i][:], wts[s][:],
                                    x_sb[s][:, NC * (cp + i):NC * (cp + i + 1)],
                                    start=(s == 0), stop=(s == KSUB - 1))
                    return fn

                def evac_fn():
                    for i in range(2):
                        dcol = slice(NC * (cp + i), NC * (cp + i + 1))
                        if evac == 'scalar':
                            nc.scalar.copy(dst[:, dcol], box['ps'][i][:])
                        else:
                            nc.vector.tensor_copy(dst[:, dcol], box['ps'][i][:])
                    sw = ropep.tile([128, 2 * NC], BF16, tag='sw')
                    box['sw'] = sw
                    for blk in range(4):
                        src = (blk // 2) * 64 + (1 - blk % 2) * 32
                        nc.sync.dma_start(sw[32 * blk:32 * (blk + 1), :],
                                          dst[src:src + 32, cs2])

                def rope_fn():
                    t1 = ropep.tile([128, 2 * NC], BF16, tag='t1')
                    nc.vector.tensor_mul(t1[:], dst[:, cs2], cos_sb[:, cs2])
                    t2 = ropep.tile([128, 2 * NC], BF16, tag='t2')
                    nc.gpsimd.tensor_tensor(t2[:], box['sw'][:],
                                            sin_sb[:, cs2], MUL)
                    nc.vector.tensor_add(dst[:, cs2], t1[:], t2[:])

                return [(1100, mk_mm(0)), (1100, mk_mm(2)), (1100, mk_mm(4)),
                        (1100, mk_mm(6)), (700, evac_fn), (900, rope_fn)]

            def v_item(t, evac):
                """V projection for token tile t (all 8 heads)."""
                box = {}

                def a():
                    box['ps'] = projps.tile([128, NC], F32, tag='proj',
                                            name=f'pv{t}')
                    for s in range(4):
                        nc.tensor.matmul(
                            box['ps'][:], x_sb[s][:, 128 * t:128 * (t + 1)],
                            wv_tiles[s][:], start=(s == 0), stop=False)

                def b():
                    for s in range(4, KSUB):
                        nc.tensor.matmul(
                            box['ps'][:], x_sb[s][:, 128 * t:128 * (t + 1)],
                            wv_tiles[s][:], start=False, stop=(s == KSUB - 1))
                    r = box['ps'].rearrange('p (h m) -> p h m', h=NHC)
                    if evac == 'scalar':
                        nc.scalar.copy(v_sb[:, t], r)
                    else:
                        nc.vector.tensor_copy(v_sb[:, t], r)

                return [(1100, a), (1400, b)]

            def wo_item(t):
                """Output projection for token tile t (both halves)."""
                ts_ = slice(128 * t, 128 * (t + 1))
                box = {}

                def a():
                    box['ps'] = [projps.tile([128, NC], F32, tag='proj',
                                             name=f'pwo{t}_{i}')
                                 for i in range(2)]
                    for g in range(2):
                        for half in range(2):
                            nc.tensor.matmul(
                                box['ps'][half][:], lhs_sb[:, g, ts_],
                                wo_sb[:, g, NC * half:NC * (half + 1)],
                                start=(g == 0), stop=False)

                def b():
                    for g in range(2, NG):
                        for half in range(2):
                            nc.tensor.matmul(
                                box['ps'][half][:], lhs_sb[:, g, ts_],
                                wo_sb[:, g, NC * half:NC * (half + 1)],
                                start=False, stop=(g == NG - 1))
                    for half in range(2):
                        yt = ytp.tile([128, NC], BF16, tag='yt')
                        nc.vector.tensor_copy(yt[:], box['ps'][half][:])
                        nc.sync.dma_start(y[ts_, NC * half:NC * (half + 1)],
                                          yt[:])

                return [(1100, a), (1500, b)]

            # feeder fill order: early prep/V first (startup), late V tiles
            # and second-half prep land mid-kernel as attention filler
            push(('prep', 0, 0, 0), prep_item(0, 0, 0, 'scalar'))
            push(('prep', 0, 1, 0), prep_item(0, 1, 0, 'scalar'))
            for t in range(4):
                push(('v', t), v_item(t, 'scalar'))
            push(('prep', 1, 0, 0), prep_item(1, 0, 0, 'vector'))
            push(('prep', 1, 1, 0), prep_item(1, 1, 0, 'vector'))
            for t in range(4, 8):
                push(('v', t), v_item(t, 'vector'))
            push(('prep', 2, 0, 0), prep_item(2, 0, 0, 'vector'))
            push(('prep', 2, 1, 0), prep_item(2, 1, 0, 'vector'))
            push(('prep', 3, 0, 0), prep_item(3, 0, 0, 'vector'))
            push(('prep', 3, 1, 0), prep_item(3, 1, 0, 'vector'))
            for g in range(NG):
                push(('prep', g, 0, 2), prep_item(g, 0, 2, 'vector'))
                push(('prep', g, 1, 2), prep_item(g, 1, 2, 'vector'))
            for t in range(8, 16):
                push(('v', t), v_item(t, 'vector'))

            # ---------- attention ----------
            def emit_att_chunk(g, c):
                qt, kt_ = qk_tiles[g]
                hA, hB = 2 * g, 2 * g + 1
                cs = slice(NC * c, NC * (c + 1))
                pv = pvps.tile([128, NC], F32, tag='pv')
                den = denps.tile([128, NC], F32, tag='den')
                n_kt = 4 * (c + 1)
                p_tiles = {}

                def emit_sc(i):
                    drain(('v', i))
                    j = i - 4 * c
                    v0 = max(j, 0) * 128
                    w = NC - v0
                    qs = slice(NC * c + v0, NC * (c + 1))
                    ks = slice(128 * i, 128 * (i + 1))
                    sc = scps.tile([128, 2 * NC], F32, tag='sc')
                    nc.tensor.matmul(sc[:, v0:NC], kt_[0:64, ks],
                                     qt[0:64, qs], start=True, stop=True)
                    nc.tensor.matmul(sc[:, NC + v0:2 * NC], kt_[64:128, ks],
                                     qt[64:128, qs], start=True, stop=True)
                    p = pp.tile([128, 2 * NC], BF16, tag='p')
                    p_tiles[i] = p
                    sc_seg = bass.AP(tensor=sc.tensor, offset=sc.offset + v0,
                                     ap=[list(sc.ap[0]), [NC, 2], [1, w]])
                    p_seg = bass.AP(tensor=p.tensor, offset=p.offset + v0,
                                    ap=[list(p.ap[0]), [NC, 2], [1, w]])
                    nc.scalar.activation(out=p_seg, in_=sc_seg, func=EXP,
                                         scale=1.0 / np.sqrt(DK))
                    if j >= 0:       # triangle on the diag sub-block
                        dseg = bass.AP(
                            tensor=p.tensor, offset=p.offset + v0,
                            ap=[list(p.ap[0]), [NC, 2], [1, 128]])
                        nc.vector.tensor_tensor(
                            dseg, dseg,
                            tri_sb[:, None, :].to_broadcast([128, 2, 128]),
                            MUL)

                def emit_pv(i):
                    v0 = max(i - 4 * c, 0) * 128
                    sA = slice(v0, NC)
                    sB = slice(NC + v0, 2 * NC)
                    st_, sp_ = (i == 0), (i == n_kt - 1)
                    p = p_tiles[i]
                    nc.tensor.matmul(pv[0:64, sA], v_sb[:, i, hA, :],
                                     p[:, sA], start=st_, stop=sp_)
                    nc.tensor.matmul(pv[64:128, sA], v_sb[:, i, hB, :],
                                     p[:, sB], start=st_, stop=sp_)

                def emit_den2(i):
                    # c==0: later blocks are width-trimmed, so keep a single
                    # accumulator pair (rows 0/32) that block 0 fully covers
                    v0 = max(i - 4 * c, 0) * 128
                    sA = slice(v0, NC)
                    sB = slice(NC + v0, 2 * NC)
                    st_, sp_ = (i == 0), (i == n_kt - 1)
                    p = p_tiles.pop(i)
                    nc.tensor.matmul(den[0:1, sA], ones_sb[:], p[:, sA],
                                     start=st_, stop=sp_, tile_position=(0, 0))
                    nc.tensor.matmul(den[32:33, sA], ones_sb[:], p[:, sB],
                                     start=st_, stop=sp_, tile_position=(0, 32))

                def emit_den4(m):
                    # kt pair (2m, 2m+1): 4 ones-matmuls packed across PE
                    # column groups run concurrently -> one w-col pass per pair
                    st_, sp_ = (m == 0), (m == n_kt // 2 - 1)
                    for idx, i in enumerate((2 * m, 2 * m + 1)):
                        v0 = max(i - 4 * c, 0) * 128
                        sA = slice(v0, NC)
                        sB = slice(NC + v0, 2 * NC)
                        p = p_tiles.pop(i)
                        ro = 64 * idx
                        nc.tensor.matmul(den[ro:ro + 1, sA], ones_sb[:],
                                         p[:, sA], start=st_, stop=sp_,
                                         tile_position=(0, ro))
                        nc.tensor.matmul(den[ro + 32:ro + 33, sA], ones_sb[:],
                                         p[:, sB], start=st_, stop=sp_,
                                         tile_position=(0, ro + 32))

                for i in range(n_kt):
                    emit_sc(i)
                    pull(state['ns'] / max(state['kts'], 1))
                    state['kts'] -= 1
                    if i >= 1:
                        emit_pv(i - 1)
                        if c == 0:
                            emit_den2(i - 1)
                        elif (i - 1) % 2 == 1:
                            emit_den4((i - 1) // 2)
                emit_pv(n_kt - 1)
                if c == 0:
                    emit_den2(n_kt - 1)
                else:
                    emit_den4(n_kt // 2 - 1)

                # evac pv early (frees the single pv bank), normalize later
                nc.vector.tensor_copy(lhs_sb[:, g, cs], pv[:])

                stage = dent.tile([128, NC], F32, tag='dstage')
                hi = 33 if c == 0 else 97
                nc.vector.tensor_copy(stage[0:hi, :], den[0:hi, :])
                packed = dent.tile([128, 8], F32, tag='dpA')
                nc.sync.dma_start(packed[0:64, :], stage[0:1, :])
                nc.sync.dma_start(packed[64:128, :], stage[32:33, :])
                if c > 0:
                    packed2 = dent.tile([128, 8], F32, tag='dpB')
                    nc.sync.dma_start(packed2[0:64, :], stage[64:65, :])
                    nc.sync.dma_start(packed2[64:128, :], stage[96:97, :])
                    nc.vector.tensor_add(packed[:], packed[:], packed2[:])
                packr = dent.tile([128, 8], BF16, tag='dpackr')
                with nc.allow_low_precision(reason='softmax 1/sum in bf16'):
                    nc.vector.reciprocal(packr[:], packed[:])
                nc.sync.dma_start(recip_d[g, c], packr[:])
                rc = rct.tile([128, NC], BF16, tag='rc')
                for half in range(2):
                    nc.sync.dma_start(
                        rc[64 * half:64 * (half + 1), :],
                        bass.AP(tensor=recip_d.tensor,
                                offset=recip_d.offset
                                + ((g * N_CHUNKS + c) * 2 + half) * NC,
                                ap=[[0, 64], [1, NC]]))
                nc.vector.tensor_tensor(lhs_sb[:, g, cs],
                                        lhs_sb[:, g, cs], rc[:], MUL)

            # ------------- chunk-major pipeline -------------
            for c in range(N_CHUNKS):
                for g in range(NG):
                    cp = 0 if c < 2 else 2
                    drain(('prep', g, 0, cp))
                    drain(('prep', g, 1, cp))
                    emit_att_chunk(g, c)
                if c < N_CHUNKS - 1:
                    # spread this chunk's output projection into the next
                    # chunk's attention window
                    for t in range(4 * c, 4 * (c + 1)):
                        push(('wo', t), wo_item(t))
                else:
                    while feeder:
                        run_next()
                    for t in range(4 * c, 4 * (c + 1)):
                        for cost, fn in wo_item(t):
                            fn()
    nc.compile()
    return nc


def _host_inputs(x, Wq, Wk, Wv, Wo, token_positions):
    """Per-core input maps (host-side sharding / layout / dtype prep only)."""
    import ml_dtypes
    bf16 = ml_dtypes.bfloat16
    perm = np.empty(DK, np.int64)
    perm[0:32] = np.arange(0, DK, 2)
    perm[32:64] = np.arange(1, DK, 2)

    inv_freq = 1.0 / (THETA ** (np.arange(0, DK, 2, dtype=np.float64) / DK))  # [32]
    ang = token_positions.astype(np.float64)[None, :] * inv_freq[:, None]     # [32, S]
    cos32 = np.cos(ang).astype(np.float32)
    sin32 = np.sin(ang).astype(np.float32)
    cos128 = np.tile(cos32, (4, 1)).astype(bf16)
    sin128 = np.concatenate([-sin32, sin32, -sin32, sin32], axis=0).astype(bf16)

    tri = (np.arange(128)[None, :] >= np.arange(128)[:, None]).astype(bf16)

    in_maps = []
    for core in range(8):
        b = core // 2
        h0 = (core % 2) * NHC
        cols = slice(h0 * DK, (h0 + NHC) * DK)
        wq_s = Wq[cols, :].reshape(NHC, DK, D)[:, perm, :].reshape(HD, D)
        wk_s = Wk[cols, :].reshape(NHC, DK, D)[:, perm, :].reshape(HD, D)
        in_maps.append({
            'xT': np.ascontiguousarray(x[b].T.astype(bf16)),
            'wqT': np.ascontiguousarray(wq_s.T.astype(bf16)),
            'wkT': np.ascontiguousarray(wk_s.T.astype(bf16)),
            'wvT': np.ascontiguousarray(Wv[cols, :].T.astype(bf16)),
            'woT': np.ascontiguousarray(Wo[:, cols].T.astype(bf16)),
            'cosd': cos128, 'sind': sin128, 'maskd': tri,
        })
    return in_maps


def kernel(x, Wq, Wk, Wv, Wo, token_positions, _results_hook=None):
    if 'nc' not in _CACHED:
        _CACHED['nc'] = _build()
    nc = _CACHED['nc']
    in_maps = _host_inputs(np.asarray(x), np.asarray(Wq), np.asarray(Wk),
                           np.asarray(Wv), np.asarray(Wo),
                           np.asarray(token_positions))
    res = run_bass_kernel_spmd(nc, in_maps, list(range(8)),
                               **(_results_hook or {}))
    if _results_hook is not None:
        _CACHED['last'] = res
    out = np.empty((B, S, D), np.float32)
    for b in range(B):
        out[b] = (res.results[2 * b]['y'].astype(np.float32)
                  + res.results[2 * b + 1]['y'].astype(np.float32))
    return out


# revision 4
# speedup vs baseline: 1.1203x; 1.1147x over previous
"""Causal multi-head self-attention with RoPE on 8 Trainium2 NeuronCores.

Problem: b=4, s=2048, d_model=1024, 16 heads, dk=64, causal, RoPE(theta=1e4).

Sharding: 8 cores = (batch, head-half). Core c handles batch c//2 and heads
(c%2)*8 .. +8: QKV projections, causal attention, partial output projection;
the host sums the two partials per batch.

V3 structure: anti-diagonal wavefront over (head-pair group, q-chunk) so the
exp load on the scalar engine (the per-block pacer) is uniform across the
kernel instead of piling up in the late, feeder-poor chunks. A fine-grained
projection feeder slices Q/K/V/Wo matmuls into ~1us closures pulled into the
PE stream between attention blocks. Scores for block kt are emitted before PV
of block kt-1 (one-block softmax lookahead). Softmax denominators ride
ones-column matmuls packed 4-wide across kt pairs (PE column strips
0/32/64/96). Inputs land via one multi-dim DMA per tensor-chunk (the sync
engine pays ~0.6us per dma_start dispatch, so descriptor count >> dispatch
count); RoPE-swap / output / den-chain DMAs dispatch from the gpsimd and
vector queues to keep sync off the critical path. The exp activation table is
pre-warmed during the input DMA window.
"""
import sys
import numpy as np

for _p in ('/root/.axon_site/_ro/trn_rl_repo', '/opt/trn_rl_repo'):
    if _p not in sys.path:
        sys.path.append(_p)

import concourse.bass as bass
import concourse.tile as tile
from concourse import bacc, mybir
from concourse.bass_utils import run_bass_kernel_spmd

F32 = mybir.dt.float32
BF16 = mybir.dt.bfloat16
EXP = mybir.ActivationFunctionType.Exp
MUL = mybir.AluOpType.mult

B, S, D = 4, 2048, 1024
NH, DK = 16, 64
NHC = 8            # heads per core
HD = NHC * DK      # 512
NG = 4             # head-pairs per core
NC = 512           # q-chunk
N_CHUNKS = S // NC
N_KT = S // 128
KSUB = D // 128
THETA = 10000.0

_CACHED = {}


def _build():
    nc = bacc.Bacc('TRN2', target_bir_lowering=False, debug=False, num_devices=8)
    xT = nc.dram_tensor('xT', [D, S], BF16, kind='ExternalInput').ap()
    wqT = nc.dram_tensor('wqT', [D, HD], BF16, kind='ExternalInput').ap()
    wkT = nc.dram_tensor('wkT', [D, HD], BF16, kind='ExternalInput').ap()
    wvT = nc.dram_tensor('wvT', [D, HD], BF16, kind='ExternalInput').ap()
    woT = nc.dram_tensor('woT', [HD, D], BF16, kind='ExternalInput').ap()
    cosd = nc.dram_tensor('cosd', [128, S], BF16, kind='ExternalInput').ap()
    sind = nc.dram_tensor('sind', [128, S], BF16, kind='ExternalInput').ap()
    maskd = nc.dram_tensor('maskd', [128, 128], BF16, kind='ExternalInput').ap()
    y = nc.dram_tensor('y', [S, D], BF16, kind='ExternalOutput').ap()
    recip_d = nc.dram_tensor('recip_d', [NG, N_CHUNKS, 2, NC], BF16).ap()

    with tile.TileContext(nc) as tc:
        with tc.tile_pool(name='persist', bufs=1) as persist, \
             tc.tile_pool(name='rope', bufs=2) as ropep, \
             tc.tile_pool(name='pp', bufs=6) as pp, \
             tc.tile_pool(name='dent', bufs=2) as dent, \
             tc.tile_pool(name='rct', bufs=2) as rct, \
             tc.tile_pool(name='ytp', bufs=2) as ytp, \
             tc.tile_pool(name='scps', bufs=2, space='PSUM') as scps, \
             tc.tile_pool(name='pvps', bufs=1, space='PSUM') as pvps, \
             tc.tile_pool(name='denps', bufs=1, space='PSUM') as denps, \
             tc.tile_pool(name='projps', bufs=2, space='PSUM') as projps:

            # ---------- persistent tiles; one multi-dim DMA per chunk ------
            wqk = {}

            def load_w(g, which):
                w_ap = wqT if which == 0 else wkT
                nm = 'wq' if which == 0 else 'wk'
                wt = persist.tile([128, KSUB, 128], BF16, tag=f'{nm}{g}',
                                  name=f'{nm}{g}')
                src = bass.AP(tensor=w_ap.tensor, offset=128 * g,
                              ap=[[HD, 128], [128 * HD, KSUB], [1, 128]])
                nc.sync.dma_start(wt[:], src)
                wqk[(g, which)] = wt

            x_sb = persist.tile([128, KSUB, S], BF16, tag='x_sb')
            cos_sb = persist.tile([128, S], BF16, tag='cos_sb')
            sin_sb = persist.tile([128, S], BF16, tag='sin_sb')
            tri_sb = persist.tile([128, 128], BF16, tag='tri_sb')
            ones_sb = persist.tile([128, 1], BF16, tag='ones_sb')
            v_sb = persist.tile([128, N_KT, NHC, DK], BF16, tag='v_sb')
            lhs_sb = persist.tile([128, NG, S], BF16, tag='lhs_sb')
            wo_sb = persist.tile([128, NG, D], BF16, tag='wo_sb')
            wv_sb = persist.tile([128, KSUB, HD], BF16, tag='wv_sb')

            def dma_x(c):
                src = bass.AP(tensor=xT.tensor, offset=NC * c,
                              ap=[[S, 128], [128 * S, KSUB], [1, NC]])
                nc.sync.dma_start(x_sb[:, :, NC * c:NC * (c + 1)], src)

            # startup-critical first
            load_w(0, 0)
            load_w(0, 1)
            dma_x(0)
            dma_x(1)
            nc.sync.dma_start(
                wv_sb[:], bass.AP(tensor=wvT.tensor, offset=0,
                                  ap=[[HD, 128], [128 * HD, KSUB], [1, HD]]))
            nc.sync.dma_start(cos_sb[:, 0:2 * NC], cosd[:, 0:2 * NC])
            nc.sync.dma_start(sin_sb[:, 0:2 * NC], sind[:, 0:2 * NC])
            nc.sync.dma_start(tri_sb[:], maskd)
            nc.vector.memset(ones_sb[:], 1.0)
            # pre-warm the exp activation table during the DMA window
            wrm = dent.tile([128, 8], BF16, tag='warm')
            wrm2 = dent.tile([128, 8], BF16, tag='warm2')
            nc.vector.memset(wrm[:], 0.0)
            nc.scalar.activation(out=wrm2[:], in_=wrm[:], func=EXP, scale=1.0)
            load_w(1, 0)
            load_w(1, 1)
            dma_x(2)
            dma_x(3)
            nc.sync.dma_start(cos_sb[:, 2 * NC:S], cosd[:, 2 * NC:S])
            nc.sync.dma_start(sin_sb[:, 2 * NC:S], sind[:, 2 * NC:S])
            load_w(2, 0)
            load_w(2, 1)
            load_w(3, 0)
            load_w(3, 1)
            nc.sync.dma_start(
                wo_sb[:], bass.AP(tensor=woT.tensor, offset=0,
                                  ap=[[D, 128], [128 * D, NG], [1, D]]))

            qk_tiles = {}
            for g in range(NG):
                qk_tiles[g] = [
                    persist.tile([128, S], BF16, tag=f'q{g}', name=f'q{g}'),
                    persist.tile([128, S], BF16, tag=f'k{g}', name=f'k{g}')]

            # ---------- feeder: projection work sliced into ~1us closures ----
            feeder = []          # list of [tag_or_None, cost_ns, closure]
            done_tags = set()
            state = {'ns': 0.0,
                     'kts': sum(4 * (c + 1) for c in range(N_CHUNKS)) * NG}

            def push(tag, closures):
                for i, (cost, fn) in enumerate(closures):
                    t = tag if i == len(closures) - 1 else None
                    feeder.append((t, cost, fn))
                    state['ns'] += cost

            def run_next():
                t, cost, fn = feeder.pop(0)
                fn()
                state['ns'] -= cost
                if t is not None:
                    done_tags.add(t)

            def drain(tag):
                while tag not in done_tags:
                    assert feeder, f'feeder empty before {tag}'
                    run_next()

            def pull(budget):
                while feeder and budget > 0:
                    budget -= feeder[0][1]
                    run_next()

            def prep_item(g, which, cp, evac):
                """Q/K projection + RoPE for chunks (cp, cp+1) of group g."""
                wt = wqk[(g, which)]
                dst = qk_tiles[g][which]
                cs2 = slice(NC * cp, NC * (cp + 2))
                box = {}

                def mk_mm(s0):
                    def fn():
                        if s0 == 0:
                            box['ps'] = [
                                projps.tile([128, NC], F32, tag='proj',
                                            name=f'pqk{g}_{which}_{cp}_{i}')
                                for i in range(2)]
                        for s in (s0, s0 + 1):
                            for i in range(2):
                                nc.tensor.matmul(
                                    box['ps'][i][:], wt[:, s, :],
                                    x_sb[:, s, NC * (cp + i):NC * (cp + i + 1)],
                                    start=(s == 0), stop=(s == KSUB - 1))
                    return fn

                def evac_fn():
                    for i in range(2):
                        dcol = slice(NC * (cp + i), NC * (cp + i + 1))
                        if evac == 'scalar':
                            nc.scalar.copy(dst[:, dcol], box['ps'][i][:])
                        else:
                            nc.vector.tensor_copy(dst[:, dcol], box['ps'][i][:])
                    sw = ropep.tile([128, 2 * NC], BF16, tag='sw')
                    box['sw'] = sw
                    for blk in range(4):
                        src = (blk // 2) * 64 + (1 - blk % 2) * 32
                        nc.gpsimd.dma_start(sw[32 * blk:32 * (blk + 1), :],
                                            dst[src:src + 32, cs2])

                def rope_fn():
                    t1 = ropep.tile([128, 2 * NC], BF16, tag='t1')
                    nc.vector.tensor_mul(t1[:], dst[:, cs2], cos_sb[:, cs2])
                    t2 = ropep.tile([128, 2 * NC], BF16, tag='t2')
                    nc.gpsimd.tensor_tensor(t2[:], box['sw'][:],
                                            sin_sb[:, cs2], MUL)
                    nc.vector.tensor_add(dst[:, cs2], t1[:], t2[:])

                return [(1100, mk_mm(0)), (1100, mk_mm(2)), (1100, mk_mm(4)),
                        (1100, mk_mm(6)), (700, evac_fn), (900, rope_fn)]

            def v_item(t, evac):
                """V projection for token tile t (all 8 heads)."""
                box = {}

                def a():
                    box['ps'] = projps.tile([128, NC], F32, tag='proj',
                                            name=f'pv{t}')
                    for s in range(4):
                        nc.tensor.matmul(
                            box['ps'][:], x_sb[:, s, 128 * t:128 * (t + 1)],
                            wv_sb[:, s, :], start=(s == 0), stop=False)

                def b():
                    for s in range(4, KSUB):
                        nc.tensor.matmul(
                            box['ps'][:], x_sb[:, s, 128 * t:128 * (t + 1)],
                            wv_sb[:, s, :], start=False, stop=(s == KSUB - 1))
                    r = box['ps'].rearrange('p (h m) -> p h m', h=NHC)
                    if evac == 'scalar':
                        nc.scalar.copy(v_sb[:, t], r)
                    else:
                        nc.vector.tensor_copy(v_sb[:, t], r)

                return [(1100, a), (1400, b)]

            def wo_item(t):
                """Output projection for token tile t (both halves)."""
                ts_ = slice(128 * t, 128 * (t + 1))
                box = {}

                def a():
                    box['ps'] = [projps.tile([128, NC], F32, tag='proj',
                                             name=f'pwo{t}_{i}')
                                 for i in range(2)]
                    for g in range(2):
                        for half in range(2):
                            nc.tensor.matmul(
                                box['ps'][half][:], lhs_sb[:, g, ts_],
                                wo_sb[:, g, NC * half:NC * (half + 1)],
                                start=(g == 0), stop=False)

                def b():
                    for g in range(2, NG):
                        for half in range(2):
                            nc.tensor.matmul(
                                box['ps'][half][:], lhs_sb[:, g, ts_],
                                wo_sb[:, g, NC * half:NC * (half + 1)],
                                start=False, stop=(g == NG - 1))
                    yt = ytp.tile([128, D], BF16, tag='yt')
                    for half in range(2):
                        nc.vector.tensor_copy(yt[:, NC * half:NC * (half + 1)],
                                              box['ps'][half][:])
                    nc.sync.dma_start(y[ts_, :], yt[:])

                return [(1100, a), (1500, b)]

            # feeder fill order follows wavefront consumption order
            push(('prep', 0, 0, 0), prep_item(0, 0, 0, 'scalar'))
            push(('prep', 0, 1, 0), prep_item(0, 1, 0, 'scalar'))
            for t in range(4):
                push(('v', t), v_item(t, 'scalar'))
            push(('prep', 1, 0, 0), prep_item(1, 0, 0, 'vector'))
            push(('prep', 1, 1, 0), prep_item(1, 1, 0, 'vector'))
            for t in range(4, 8):
                push(('v', t), v_item(t, 'vector'))
            push(('prep', 0, 0, 2), prep_item(0, 0, 2, 'vector'))
            push(('prep', 0, 1, 2), prep_item(0, 1, 2, 'vector'))
            push(('prep', 2, 0, 0), prep_item(2, 0, 0, 'vector'))
            push(('prep', 2, 1, 0), prep_item(2, 1, 0, 'vector'))
            for t in range(8, 12):
                push(('v', t), v_item(t, 'vector'))
            push(('prep', 1, 0, 2), prep_item(1, 0, 2, 'vector'))
            push(('prep', 1, 1, 2), prep_item(1, 1, 2, 'vector'))
            push(('prep', 3, 0, 0), prep_item(3, 0, 0, 'vector'))
            push(('prep', 3, 1, 0), prep_item(3, 1, 0, 'vector'))
            for t in range(12, 16):
                push(('v', t), v_item(t, 'vector'))
            push(('prep', 2, 0, 2), prep_item(2, 0, 2, 'vector'))
            push(('prep', 2, 1, 2), prep_item(2, 1, 2, 'vector'))
            push(('prep', 3, 0, 2), prep_item(3, 0, 2, 'vector'))
            push(('prep', 3, 1, 2), prep_item(3, 1, 2, 'vector'))

            # ---------- attention ----------
            def emit_att_chunk(g, c):
                qt, kt_ = qk_tiles[g]
                hA, hB = 2 * g, 2 * g + 1
                cs = slice(NC * c, NC * (c + 1))
                pv = pvps.tile([128, NC], F32, tag='pv')
                den = denps.tile([128, NC], F32, tag='den')
                n_kt = 4 * (c + 1)
                p_tiles = {}

                def emit_sc(i):
                    drain(('v', i))
                    j = i - 4 * c
                    v0 = max(j, 0) * 128
                    w = NC - v0
                    qs = slice(NC * c + v0, NC * (c + 1))
                    ks = slice(128 * i, 128 * (i + 1))
                    sc = scps.tile([128, 2 * NC], F32, tag='sc')
                    nc.tensor.matmul(sc[:, v0:NC], kt_[0:64, ks],
                                     qt[0:64, qs], start=True, stop=True)
                    nc.tensor.matmul(sc[:, NC + v0:2 * NC], kt_[64:128, ks],
                                     qt[64:128, qs], start=True, stop=True)
                    p = pp.tile([128, 2 * NC], BF16, tag='p')
                    p_tiles[i] = p
                    sc_seg = bass.AP(tensor=sc.tensor, offset=sc.offset + v0,
                                     ap=[list(sc.ap[0]), [NC, 2], [1, w]])
                    p_seg = bass.AP(tensor=p.tensor, offset=p.offset + v0,
                                    ap=[list(p.ap[0]), [NC, 2], [1, w]])
                    nc.scalar.activation(out=p_seg, in_=sc_seg, func=EXP,
                                         scale=1.0 / np.sqrt(DK))
                    if j >= 0:       # triangle on the diag sub-block
                        dseg = bass.AP(
                            tensor=p.tensor, offset=p.offset + v0,
                            ap=[list(p.ap[0]), [NC, 2], [1, 128]])
                        nc.vector.tensor_tensor(
                            dseg, dseg,
                            tri_sb[:, None, :].to_broadcast([128, 2, 128]),
                            MUL)

                def emit_pv(i):
                    v0 = max(i - 4 * c, 0) * 128
                    sA = slice(v0, NC)
                    sB = slice(NC + v0, 2 * NC)
                    st_, sp_ = (i == 0), (i == n_kt - 1)
                    p = p_tiles[i]
                    nc.tensor.matmul(pv[0:64, sA], v_sb[:, i, hA, :],
                                     p[:, sA], start=st_, stop=sp_)
                    nc.tensor.matmul(pv[64:128, sA], v_sb[:, i, hB, :],
                                     p[:, sB], start=st_, stop=sp_)

                def emit_den2(i):
                    # c==0: later blocks are width-trimmed, so keep a single
                    # accumulator pair (rows 0/32) that block 0 fully covers
                    v0 = max(i - 4 * c, 0) * 128
                    sA = slice(v0, NC)
                    sB = slice(NC + v0, 2 * NC)
                    st_, sp_ = (i == 0), (i == n_kt - 1)
                    p = p_tiles.pop(i)
                    nc.tensor.matmul(den[0:1, sA], ones_sb[:], p[:, sA],
                                     start=st_, stop=sp_, tile_position=(0, 0))
                    nc.tensor.matmul(den[32:33, sA], ones_sb[:], p[:, sB],
                                     start=st_, stop=sp_, tile_position=(0, 32))

                def emit_den4(m):
                    # kt pair (2m, 2m+1): 4 ones-matmuls packed across PE
                    # column strips run concurrently -> one w-col pass per pair
                    st_, sp_ = (m == 0), (m == n_kt // 2 - 1)
                    for idx, i in enumerate((2 * m, 2 * m + 1)):
                        v0 = max(i - 4 * c, 0) * 128
                        sA = slice(v0, NC)
                        sB = slice(NC + v0, 2 * NC)
                        p = p_tiles.pop(i)
                        ro = 64 * idx
                        nc.tensor.matmul(den[ro:ro + 1, sA], ones_sb[:],
                                         p[:, sA], start=st_, stop=sp_,
                                         tile_position=(0, ro))
                        nc.tensor.matmul(den[ro + 32:ro + 33, sA], ones_sb[:],
                                         p[:, sB], start=st_, stop=sp_,
                                         tile_position=(0, ro + 32))

                for i in range(n_kt):
                    emit_sc(i)
                    pull(state['ns'] / max(state['kts'], 1))
                    state['kts'] -= 1
                    if i >= 1:
                        emit_pv(i - 1)
                        if c == 0:
                            emit_den2(i - 1)
                        elif (i - 1) % 2 == 1:
                            emit_den4((i - 1) // 2)
                emit_pv(n_kt - 1)
                if c == 0:
                    emit_den2(n_kt - 1)
                else:
                    emit_den4(n_kt // 2 - 1)

                # evac pv early (frees the single pv bank), normalize later
                nc.vector.tensor_copy(lhs_sb[:, g, cs], pv[:])

                stage = dent.tile([128, NC], F32, tag='dstage')
                hi = 33 if c == 0 else 97
                nc.vector.tensor_copy(stage[0:hi, :], den[0:hi, :])
                packed = dent.tile([128, 8], F32, tag='dpA')
                nc.sync.dma_start(packed[0:64, :], stage[0:1, :])
                nc.sync.dma_start(packed[64:128, :], stage[32:33, :])
                if c > 0:
                    packed2 = dent.tile([128, 8], F32, tag='dpB')
                    nc.sync.dma_start(packed2[0:64, :], stage[64:65, :])
                    nc.sync.dma_start(packed2[64:128, :], stage[96:97, :])
                    nc.vector.tensor_add(packed[:], packed[:], packed2[:])
                packr = dent.tile([128, 8], BF16, tag='dpackr')
                with nc.allow_low_precision(reason='softmax 1/sum in bf16'):
                    nc.vector.reciprocal(packr[:], packed[:])
                nc.sync.dma_start(recip_d[g, c], packr[:])
                rc = rct.tile([128, NC], BF16, tag='rc')
                for half in range(2):
                    nc.sync.dma_start(
                        rc[64 * half:64 * (half + 1), :],
                        bass.AP(tensor=recip_d.tensor,
                                offset=recip_d.offset
                                + ((g * N_CHUNKS + c) * 2 + half) * NC,
                                ap=[[0, 64], [1, NC]]))
                nc.vector.tensor_tensor(lhs_sb[:, g, cs],
                                        lhs_sb[:, g, cs], rc[:], MUL)

            # ------------- anti-diagonal wavefront -------------
            done_cnt = [0] * N_CHUNKS
            for k in range(N_CHUNKS + NG - 1):
                for g in range(max(0, k - N_CHUNKS + 1), min(NG - 1, k) + 1):
                    c = k - g
                    cp = 0 if c < 2 else 2
                    drain(('prep', g, 0, cp))
                    drain(('prep', g, 1, cp))
                    emit_att_chunk(g, c)
                    done_cnt[c] += 1
                    if done_cnt[c] == NG:
                        if c < N_CHUNKS - 1:
                            for t in range(4 * c, 4 * (c + 1)):
                                push(('wo', t), wo_item(t))
                        else:
                            while feeder:
                                run_next()
                            for t in range(4 * c, 4 * (c + 1)):
                                for cost, fn in wo_item(t):
                                    fn()
    nc.compile()
    return nc


def _host_inputs(x, Wq, Wk, Wv, Wo, token_positions):
    """Per-core input maps (host-side sharding / layout / dtype prep only)."""
    import ml_dtypes
    bf16 = ml_dtypes.bfloat16
    perm = np.empty(DK, np.int64)
    perm[0:32] = np.arange(0, DK, 2)
    perm[32:64] = np.arange(1, DK, 2)

    inv_freq = 1.0 / (THETA ** (np.arange(0, DK, 2, dtype=np.float64) / DK))  # [32]
    ang = token_positions.astype(np.float64)[None, :] * inv_freq[:, None]     # [32, S]
    cos32 = np.cos(ang).astype(np.float32)
    sin32 = np.sin(ang).astype(np.float32)
    cos128 = np.tile(cos32, (4, 1)).astype(bf16)
    sin128 = np.concatenate([-sin32, sin32, -sin32, sin32], axis=0).astype(bf16)

    tri = (np.arange(128)[None, :] >= np.arange(128)[:, None]).astype(bf16)

    in_maps = []
    for core in range(8):
        b = core // 2
        h0 = (core % 2) * NHC
        cols = slice(h0 * DK, (h0 + NHC) * DK)
        wq_s = Wq[cols, :].reshape(NHC, DK, D)[:, perm, :].reshape(HD, D)
        wk_s = Wk[cols, :].reshape(NHC, DK, D)[:, perm, :].reshape(HD, D)
        in_maps.append({
            'xT': np.ascontiguousarray(x[b].T.astype(bf16)),
            'wqT': np.ascontiguousarray(wq_s.T.astype(bf16)),
            'wkT': np.ascontiguousarray(wk_s.T.astype(bf16)),
            'wvT': np.ascontiguousarray(Wv[cols, :].T.astype(bf16)),
            'woT': np.ascontiguousarray(Wo[:, cols].T.astype(bf16)),
            'cosd': cos128, 'sind': sin128, 'maskd': tri,
        })
    return in_maps


def kernel(x, Wq, Wk, Wv, Wo, token_positions, _results_hook=None):
    if 'nc' not in _CACHED:
        _CACHED['nc'] = _build()
    nc = _CACHED['nc']
    in_maps = _host_inputs(np.asarray(x), np.asarray(Wq), np.asarray(Wk),
                           np.asarray(Wv), np.asarray(Wo),
                           np.asarray(token_positions))
    res = run_bass_kernel_spmd(nc, in_maps, list(range(8)),
                               **(_results_hook or {}))
    if _results_hook is not None:
        _CACHED['last'] = res
    out = np.empty((B, S, D), np.float32)
    for b in range(B):
        out[b] = (res.results[2 * b]['y'].astype(np.float32)
                  + res.results[2 * b + 1]['y'].astype(np.float32))
    return out
